# revision 21
# baseline (speedup 1.0000x reference)
"""Causal self-attention with RoPE (B=2, T=2048, C=1024, H=16, D=64) on 8
Trainium2 NeuronCores.

Sharding: tensor-parallel over heads — each core owns 2 heads (QKV and output
projections sliced on the head axis); the per-core partial outputs (full
[C, B*T] each) are summed on the host.

v3: engine-op-count-driven schedule. All matmuls bf16. QKV streams E/O
stationaries over g-pairs (halves their LDWEIGHTS). RoPE does two merged
[128,2,TC] PSUM multiplies against a 4-way [cos|sin|sin|cos] table plus the
8-way scatter add/subs (q on DVE, k on gpsimd). One exp per (i,j) covers both
heads. V transposed on the PE into a single [128,4x128] bf16 PSUM tile,
scattered with one 4D DVE copy per chunk. Softmax normalization is a single
tensor_tensor divide per head. Out-projection computes cc-pairs into
[128,2,TC] PSUM tiles, one drain copy each (DVE/ACT split), one [256,TC] DMA
each. qkv(b=1) is emitted interleaved into attention(b=0) so its DVE-heavy
RoPE overlaps attention's PE-heavy stretch.

Per-core layout (everything transposed: features on partitions, tokens free):
  xT [1024, 4096]     x^T, shared by all cores (bf16)
  QKV proj            qkv^T chunks via PE matmul (bf16), W rows pre-permuted
                      on host into three 128-row groups:
                        E = [q_h0_even(32) | q_h1_even | k_h0_even | k_h1_even]
                        O = same rows, odd dims
                        V = [v_h0(64) | v_h1(64)]
  RoPE                m1 = [E*c; O*s], m2 = [E*s; O*c]; rot pieces scattered
                      into head-contiguous q_t/k_t [128, T] with row layout
                      [h0_e(32) | h0_o(32) | h1_e(32) | h1_o(32)]
  scores^T            S^T[kj,qi] = k_t[h].T @ q_t[h], K=64, into a merged
                      [128, 2, TC] PSUM tile (diag chunks sliced)
  softmax             exp on ScalarE (scale=1/8 folded), causal triangle via
                      gpsimd affine_select, col sums via 64 ones-columns in v
  PV                  y^T[d,qi] accumulated over kj chunks; y = num/den via
                      one DVE divide per head
  out proj            outT[c,t] partial = woutT . y^T, bf16, cc-pair DMAs

Host gathers the 8 partial outT [1024, 4096] tensors, sums, transposes.
"""

import sys
import types

import numpy as np

import concourse.bass as bass
import concourse.tile as tile
from concourse import bacc
from concourse import mybir
from concourse.bass_utils import run_bass_kernel_spmd
from concourse.masks import make_identity

F32 = mybir.dt.float32
BF16 = mybir.dt.bfloat16

B = 2
T = 2048
C = 1024
D = 64
N_CORES = 8
BT = B * T              # 4096
TC = 512                # token chunk (free dim of most matmuls)
NQI = T // TC           # 4 qi chunks per batch
NKJ = T // 128          # 16 kj chunks per batch
KC = C // 128           # 8 contraction chunks for the projections
VB = 256                # v_all cols per kj block: h0 (d|ones) | h1 (d|ones)


def _install_ntff_hook():
    """bass_utils imports antenv.axon_hooks when tracing; this image lacks it.
    Recreate it from the ctypes NTFF driver so trace=True works."""
    if "antenv.axon_hooks" in sys.modules:
        return
    try:
        from trn_agent_boot.trn_boot import _ntff_profile_via_ctypes

        hook = _ntff_profile_via_ctypes("/opt/axon/libaxon_pjrt.so")
    except Exception:
        hook = None
    mod = types.ModuleType("antenv.axon_hooks")
    mod.get_axon_ntff_profile_hook = lambda: hook
    mod.set_axon_ntff_profile_hook = lambda h: None
    sys.modules["antenv.axon_hooks"] = mod


_install_ntff_hook()

PIPE_DEPTH = 1
ROPE_K_ENGINE = "gpsimd"   # "gpsimd" | "vector"
Y_DIVIDE = False           # divide needs 2 PSUM reads (illegal); recip+mul
N_OCOPY_DVE = 2            # of 4 out-proj pair copies per g on DVE (rest ACT)


def build_nc():
    nc = bacc.Bacc(None, target_bir_lowering=False, debug=False)

    xt = nc.declare_dram_parameter("xt", [128, (BT // TC) * KC * TC], BF16, isOutput=False)
    wqkv = nc.declare_dram_parameter("wqkv", [128, KC * 384], BF16, isOutput=False)
    wout = nc.declare_dram_parameter("wout", [128, C], BF16, isOutput=False)
    cs = nc.declare_dram_parameter("cs", [128, 4 * T], BF16, isOutput=False)
    outT = nc.declare_dram_parameter("outT", [C, BT], BF16, isOutput=True)

    with tile.TileContext(nc) as tc:
        with (
            tc.sbuf_pool(name="statics", bufs=1) as statics,
            tc.sbuf_pool(name="pool_x", bufs=3) as pool_x,
            tc.sbuf_pool(name="pool_rope", bufs=2) as pool_rope,
            tc.sbuf_pool(name="pool_qk", bufs=2) as pool_qk,
            tc.sbuf_pool(name="pool_v", bufs=2) as pool_v,
            tc.sbuf_pool(name="pool_y", bufs=2) as pool_y,
            tc.sbuf_pool(name="pool_vs", bufs=2) as pool_vs,
            tc.sbuf_pool(name="pool_p", bufs=4) as pool_p,
            tc.sbuf_pool(name="pool_o", bufs=4) as pool_o,
            tc.sbuf_pool(name="pool_rb", bufs=2) as pool_rb,
            tc.psum_pool(name="ps_mm", bufs=2) as ps_mm,
            tc.psum_pool(name="ps_st", bufs=2) as ps_st,
            tc.psum_pool(name="ps_y", bufs=2) as ps_y,
        ):
            ident = statics.tile([128, 128], BF16)
            make_identity(nc, ident)

            wqkv_sb = statics.tile([128, KC * 384], BF16)
            # split so the first chunks' matmuls aren't queued behind the
            # whole 0.8 MB table
            for q in range(4):
                qs = slice(q * 2 * 384, (q + 1) * 2 * 384)
                nc.sync.dma_start(out=wqkv_sb[:, qs], in_=wqkv[:, qs])
            # deferred statics: emitted after the first xt chunk's DMA so the
            # first QKV matmuls aren't queued behind the table loads
            wout_sb = statics.tile([128, C], BF16)
            cs_sb = statics.tile([128, 4, T], BF16)
            statics_emitted = []

            def emit_deferred_statics():
                if statics_emitted:
                    return
                statics_emitted.append(1)
                csv = cs[:, :].rearrange("p (f t) -> p f t", t=T)
                # column-chunked so rope(tci) only waits for its own chunk
                for tci in range(4):
                    sl = slice(tci * TC, (tci + 1) * TC)
                    nc.sync.dma_start(out=cs_sb[:, :, sl], in_=csv[:, :, sl])
                nc.sync.dma_start(out=wout_sb, in_=wout[:, :])

            def qkv_pair(b, pair, q_t, k_t, v_all):
                """QKV + RoPE for token chunks g0 = 4b+2*pair, g1 = g0+1.
                E/O stationaries stream both chunks per LDWEIGHTS."""
                va4 = v_all.rearrange("p (n h c) -> p n h c", h=2, c=128)
                g0 = 4 * b + 2 * pair
                xts = []
                for gi, g in enumerate((g0, g0 + 1)):
                    xt_sb = pool_x.tile([128, KC, TC], BF16, tag="x", name=f"xt_{g}")
                    nc.sync.dma_start(
                        out=xt_sb,
                        in_=xt[:, g * KC * TC : (g + 1) * KC * TC].rearrange(
                            "p (kc n) -> p kc n", n=TC
                        ),
                    )
                    xts.append(xt_sb)
                emit_deferred_statics()
                if pair == 0:
                    # ones columns of v_all (cols 64:128 of each head block)
                    nc.vector.memset(va4[:, :, :, 64:128], 1.0)

                # V per chunk (mm pool), then E and O streamed over the pair
                psvs = []
                for gi in range(2):
                    psv = ps_mm.tile([128, TC], F32, tag="mm", name=f"psv_{g0}_{gi}")
                    psvs.append(psv)
                    for kc in range(KC):
                        nc.tensor.matmul(
                            psv,
                            wqkv_sb[:, kc * 384 + 256 : kc * 384 + 384],
                            xts[gi][:, kc, :],
                            start=(kc == 0),
                            stop=(kc == KC - 1),
                        )
                ps_eos = [
                    ps_st.tile([128, 2, TC], F32, tag="st", name=f"eo_{g0}_{gi}")
                    for gi in range(2)
                ]
                for mi in range(2):  # 0 = E, 1 = O
                    for kc in range(KC):
                        for gi in range(2):
                            nc.tensor.matmul(
                                ps_eos[gi][:, mi, :],
                                wqkv_sb[
                                    :, kc * 384 + 128 * mi : kc * 384 + 128 * (mi + 1)
                                ],
                                xts[gi][:, kc, :],
                                start=(kc == 0),
                                stop=(kc == KC - 1),
                            )

                for gi, g in enumerate((g0, g0 + 1)):
                    tci = 2 * pair + gi
                    # v: PSUM -> SBUF bf16; PE-transpose the four 128-blocks
                    # into one [128, 4x128] bf16 PSUM tile; single 4D scatter
                    v_sb = pool_vs.tile([128, TC], BF16, tag="vs", name=f"vsb_{g}")
                    nc.scalar.activation(
                        out=v_sb, in_=psvs[gi],
                        func=mybir.ActivationFunctionType.Copy,
                    )
                    tr = ps_mm.tile([128, 4, 128], BF16, tag="mm", name=f"tr_{g}")
                    for s in range(4):
                        nc.tensor.transpose(
                            tr[:, s, :], v_sb[:, 128 * s : 128 * (s + 1)], ident
                        )
                    nc.vector.tensor_copy(
                        out=va4[:, 4 * tci : 4 * tci + 4, :, 0:64],
                        in_=tr.rearrange("p s (h c) -> p s h c", h=2),
                    )

                    # RoPE: drain E/O to SBUF with one ACT copy (frees the st
                    # slot fast; ACT is idle in this phase), then two cheap
                    # all-SBUF-bf16 multiplies m1 = [E*c; O*s], m2 = [E*s; O*c]
                    sl = slice(tci * TC, (tci + 1) * TC)
                    eo_sb = pool_rope.tile([128, 2, TC], BF16, tag="eo", name=f"eo_{g}")
                    nc.scalar.activation(
                        out=eo_sb, in_=ps_eos[gi],
                        func=mybir.ActivationFunctionType.Copy,
                    )
                    m1 = pool_rope.tile([128, 2, TC], BF16, tag="m1", name=f"m1_{g}")
                    nc.vector.tensor_mul(out=m1, in0=eo_sb, in1=cs_sb[:, 0:2, sl])
                    m2 = pool_rope.tile([128, 2, TC], BF16, tag="m2", name=f"m2_{g}")
                    nc.vector.tensor_mul(out=m2, in0=eo_sb, in1=cs_sb[:, 2:4, sl])

                    keng = nc.gpsimd if ROPE_K_ENGINE == "gpsimd" else nc.vector
                    # q_t rows [h0e|h0o|h1e|h1o]; E rows [q0e|q1e|k0e|k1e]
                    for h in range(2):
                        he = slice(32 * h, 32 * (h + 1))
                        nc.vector.tensor_sub(
                            out=q_t[64 * h : 64 * h + 32, sl],
                            in0=m1[he, 0, :], in1=m1[he, 1, :],
                        )
                        nc.vector.tensor_add(
                            out=q_t[64 * h + 32 : 64 * h + 64, sl],
                            in0=m2[he, 0, :], in1=m2[he, 1, :],
                        )
                        ke_ = slice(64 + 32 * h, 64 + 32 * (h + 1))
                        keng.tensor_sub(
                            out=k_t[64 * h : 64 * h + 32, sl],
                            in0=m1[ke_, 0, :], in1=m1[ke_, 1, :],
                        )
                        keng.tensor_add(
                            out=k_t[64 * h + 32 : 64 * h + 64, sl],
                            in0=m2[ke_, 0, :], in1=m2[ke_, 1, :],
                        )

            def attention_iter(b, i, q_t, k_t, v_all, y_t, pending_op):
                """pending_op: previous iteration's out-proj args, emitted
                after this iteration's first scores so its PSUM tiles queue
                behind ours in the st rotation (keeps the PE fed during the
                previous iteration's recip/ymul drain)."""
                nj = 4 * i + 4
                yaccs = {}
                for h in range(2):
                    yaccs[h] = ps_y.tile(
                        [128, TC], F32, tag="y", name=f"yacc_{b}_{i}_{h}"
                    )

                def st_of(j):
                    r = j - 4 * i
                    return 128 * r if r > 0 else 0

                # software-pipelined by one step: PE issues S(j),S(j),
                # PV(j-1),PV(j-1) back-to-back while exp(j) runs on ACT
                p_tiles = {}
                for j in range(nj + PIPE_DEPTH):
                    if j == 1 and pending_op is not None:
                        outproj_chunk(*pending_op)
                    if j < nj:
                        st = st_of(j)
                        r = j - 4 * i
                        ksl = slice(128 * j, 128 * (j + 1))
                        qsl = slice(TC * i + st, TC * (i + 1))
                        ps_s = ps_st.tile(
                            [128, 2, TC], F32, tag="st", name=f"s_{b}_{i}_{j}"
                        )
                        for h in range(2):
                            hs = slice(64 * h, 64 * (h + 1))
                            nc.tensor.matmul(
                                ps_s[:, h, st:], k_t[hs, ksl], q_t[hs, qsl],
                                start=True, stop=True,
                            )
                        p_sb = pool_p.tile(
                            [128, 2, TC], BF16, tag="p", name=f"p_{b}_{i}_{j}"
                        )
                        p_tiles[j] = p_sb
                        # one exp covering both heads
                        nc.scalar.activation(
                            out=p_sb[:, :, st:], in_=ps_s[:, :, st:],
                            func=mybir.ActivationFunctionType.Exp,
                            scale=0.125,
                        )
                        if r >= 0:
                            for h in range(2):
                                nc.gpsimd.affine_select(
                                    out=p_sb[:, h, st : st + 128],
                                    in_=p_sb[:, h, st : st + 128],
                                    pattern=[[1, 128]],
                                    channel_multiplier=-1,
                                    base=0,
                                    compare_op=mybir.AluOpType.is_ge,
                                    fill=0.0,
                                )
                    if j >= PIPE_DEPTH:
                        jp = j - PIPE_DEPTH
                        st = st_of(jp)
                        p_prev = p_tiles.pop(jp)
                        for h in range(2):
                            nc.tensor.matmul(
                                yaccs[h][:, st:],
                                v_all[
                                    :, VB * jp + 128 * h : VB * jp + 128 * (h + 1)
                                ],
                                p_prev[:, h, st:],
                                start=(jp == 0),
                                stop=(jp == nj - 1),
                            )
                ysl = slice(TC * i, TC * (i + 1))
                for h in range(2):
                    if Y_DIVIDE:
                        nc.vector.tensor_tensor(
                            out=y_t[64 * h : 64 * (h + 1), ysl],
                            in0=yaccs[h][0:64, :],
                            in1=yaccs[h][64:128, :],
                            op=mybir.AluOpType.divide,
                        )
                    else:
                        rb = pool_rb.tile(
                            [128, TC], F32, tag="rb", name=f"rb_{b}_{i}_{h}"
                        )
                        nc.vector.reciprocal_approx_fast(out=rb, in_=yaccs[h])
                        nc.vector.tensor_mul(
                            out=y_t[64 * h : 64 * (h + 1), ysl],
                            in0=yaccs[h][0:64, :],
                            in1=rb[64:128],
                        )
                return (b, y_t, i)

            def outproj_chunk(b, y_t, tci):
                    g = 4 * b + tci
                    for cp in range(4):
                        ps = ps_st.tile(
                            [128, 2, TC], F32, tag="st", name=f"op_{g}_{cp}"
                        )
                        for k in range(2):
                            cc = 2 * cp + k
                            nc.tensor.matmul(
                                ps[:, k, :],
                                wout_sb[:, 128 * cc : 128 * (cc + 1)],
                                y_t[:, TC * tci : TC * (tci + 1)],
                                start=True,
                                stop=True,
                            )
                        o_sb = pool_o.tile(
                            [128, 2, TC], BF16, tag="o", name=f"o_{g}_{cp}"
                        )
                        if cp < N_OCOPY_DVE:
                            nc.vector.tensor_copy(out=o_sb, in_=ps)
                        else:
                            nc.scalar.activation(
                                out=o_sb, in_=ps,
                                func=mybir.ActivationFunctionType.Copy,
                            )
                        nc.sync.dma_start(
                            out=outT[
                                256 * cp : 256 * (cp + 1), g * TC : (g + 1) * TC
                            ].rearrange("(j p) c -> p j c", j=2),
                            in_=o_sb,
                        )

            tiles = {}
            for b in range(B):
                tiles[b] = (
                    pool_qk.tile([128, T], BF16, tag="q", name=f"q_{b}"),
                    pool_qk.tile([128, T], BF16, tag="k", name=f"k_{b}"),
                    pool_v.tile([128, VB * NKJ], BF16, tag="v", name=f"v_{b}"),
                    pool_y.tile([128, T], BF16, tag="yt", name=f"y_{b}"),
                )

            q0, k0, v0, y0 = tiles[0]
            q1, k1, v1, y1 = tiles[1]
            qkv_pair(0, 0, q0, k0, v0)
            qkv_pair(0, 1, q0, k0, v0)
            # interleave b=1's projection into b=0's attention so its
            # DVE-heavy RoPE overlaps attention's PE-heavy stretch
            pend = attention_iter(0, 0, q0, k0, v0, y0, None)
            qkv_pair(1, 0, q1, k1, v1)
            pend = attention_iter(0, 1, q0, k0, v0, y0, pend)
            qkv_pair(1, 1, q1, k1, v1)
            pend = attention_iter(0, 2, q0, k0, v0, y0, pend)
            pend = attention_iter(0, 3, q0, k0, v0, y0, pend)
            for i in range(NQI):
                pend = attention_iter(1, i, q1, k1, v1, y1, pend)
            outproj_chunk(*pend)

    nc.compile()
    return nc


_NC_CACHE = None


def _get_nc():
    global _NC_CACHE
    if _NC_CACHE is None:
        _NC_CACHE = build_nc()
    return _NC_CACHE


def _host_prep(x, qkv_w, out_w):
    import ml_dtypes

    x = np.asarray(x, dtype=np.float32)
    qkv_w = np.asarray(qkv_w, dtype=np.float32)
    out_w = np.asarray(out_w, dtype=np.float32)

    # xt[p, ((g*KC)+kc)*TC + n] = x[g*TC + n, kc*128 + p] — one contiguous
    # line per (partition, chunk) for the per-chunk DMA
    xt = np.ascontiguousarray(
        x.reshape(BT // TC, TC, KC, 128).transpose(3, 0, 2, 1).reshape(128, -1)
    ).astype(ml_dtypes.bfloat16)

    # rope tables: row p uses frequency index p % 32; layout [c|s|s|c]
    t_idx = np.arange(T, dtype=np.float64)
    inv_freq = 1.0 / (10000.0 ** (np.arange(0, D, 2, dtype=np.float64) / D))  # 32
    ang = np.outer(np.tile(inv_freq, 4), t_idx)  # [128, T]
    cosa = np.cos(ang)
    sina = np.sin(ang)
    cs = np.concatenate([cosa, sina, sina, cosa], axis=1).astype(ml_dtypes.bfloat16)

    in_maps = []
    for core in range(N_CORES):
        h0 = 2 * core
        h1 = h0 + 1
        ev = np.arange(0, D, 2)
        od = np.arange(1, D, 2)
        e_rows = np.concatenate(
            [h0 * D + ev, h1 * D + ev, C + h0 * D + ev, C + h1 * D + ev]
        )
        o_rows = np.concatenate(
            [h0 * D + od, h1 * D + od, C + h0 * D + od, C + h1 * D + od]
        )
        v_rows = np.concatenate(
            [2 * C + h0 * D + np.arange(D), 2 * C + h1 * D + np.arange(D)]
        )
        rows = np.concatenate([e_rows, o_rows, v_rows])  # [384]
        w_part = qkv_w[rows]  # [384, C]
        # wqkv[p, kc*384 + m] = w_part[m, kc*128 + p]
        wqkv_c = np.ascontiguousarray(
            w_part.T.reshape(KC, 128, 384).transpose(1, 0, 2).reshape(128, KC * 384)
        ).astype(ml_dtypes.bfloat16)
        cols = np.concatenate([h0 * D + np.arange(D), h1 * D + np.arange(D)])
        wout_c = np.ascontiguousarray(out_w[:, cols].T).astype(
            ml_dtypes.bfloat16
        )  # [128, C]
        in_maps.append({"xt": xt, "wqkv": wqkv_c, "wout": wout_c, "cs": cs})
    return in_maps


def _run(in_maps, trace=False):
    nc = _get_nc()
    return run_bass_kernel_spmd(
        nc, in_maps, core_ids=list(range(N_CORES)), trace=trace
    )


def kernel(x, qkv_w, out_w, _trace=False, _results_box=None):
    in_maps = _host_prep(x, qkv_w, out_w)
    res = _run(in_maps, trace=_trace)
    if _results_box is not None:
        _results_box.append(res)
    acc = np.zeros((C, BT), np.float32)
    for r in res.results:
        acc += r["outT"].astype(np.float32)
    out = acc.T.reshape(B, T, C)
    return np.ascontiguousarray(out)


# revision 22
# speedup vs baseline: 1.2132x; 1.2132x over previous
"""Causal self-attention with RoPE (B=2, T=2048, C=1024, H=16, D=64) on 8
Trainium2 NeuronCores.

Sharding: tensor-parallel over heads — each core owns 2 heads (QKV and output
projections sliced on the head axis); the per-core partial outputs (full
[C, B*T] each) are summed on the host.

v3: engine-op-count-driven schedule. All matmuls bf16. QKV streams E/O
stationaries over g-pairs (halves their LDWEIGHTS). RoPE does two merged
[128,2,TC] PSUM multiplies against a 4-way [cos|sin|sin|cos] table plus the
8-way scatter add/subs (q on DVE, k on gpsimd). One exp per (i,j) covers both
heads. V transposed on the PE into a single [128,4x128] bf16 PSUM tile,
scattered with one 4D DVE copy per chunk. Softmax normalization is a single
tensor_tensor divide per head. Out-projection computes cc-pairs into
[128,2,TC] PSUM tiles, one drain copy each (DVE/ACT split), one [256,TC] DMA
each. qkv(b=1) is emitted interleaved into attention(b=0) so its DVE-heavy
RoPE overlaps attention's PE-heavy stretch.

Per-core layout (everything transposed: features on partitions, tokens free):
  xT [1024, 4096]     x^T, shared by all cores (bf16)
  QKV proj            qkv^T chunks via PE matmul (bf16), W rows pre-permuted
                      on host into three 128-row groups:
                        E = [q_h0_even(32) | q_h1_even | k_h0_even | k_h1_even]
                        O = same rows, odd dims
                        V = [v_h0(64) | v_h1(64)]
  RoPE                m1 = [E*c; O*s], m2 = [E*s; O*c]; rot pieces scattered
                      into head-contiguous q_t/k_t [128, T] with row layout
                      [h0_e(32) | h0_o(32) | h1_e(32) | h1_o(32)]
  scores^T            S^T[kj,qi] = k_t[h].T @ q_t[h], K=64, into a merged
                      [128, 2, TC] PSUM tile (diag chunks sliced)
  softmax             exp on ScalarE (scale=1/8 folded), causal triangle via
                      gpsimd affine_select, col sums via 64 ones-columns in v
  PV                  y^T[d,qi] accumulated over kj chunks; y = num/den via
                      one DVE divide per head
  out proj            outT[c,t] partial = woutT . y^T, bf16, cc-pair DMAs

Host gathers the 8 partial outT [1024, 4096] tensors, sums, transposes.
"""

import sys
import types

import numpy as np

import concourse.bass as bass
import concourse.tile as tile
from concourse import bacc
from concourse import mybir
from concourse.bass_utils import run_bass_kernel_spmd
from concourse.masks import make_identity

F32 = mybir.dt.float32
BF16 = mybir.dt.bfloat16

B = 2
T = 2048
C = 1024
D = 64
N_CORES = 8
BT = B * T              # 4096
TC = 512                # token chunk (free dim of most matmuls)
NQI = T // TC           # 4 qi chunks per batch
NKJ = T // 128          # 16 kj chunks per batch
KC = C // 128           # 8 contraction chunks for the projections
VB = 256                # v_all cols per kj block: h0 (d|ones) | h1 (d|ones)


def _install_ntff_hook():
    """bass_utils imports antenv.axon_hooks when tracing; this image lacks it.
    Recreate it from the ctypes NTFF driver so trace=True works."""
    if "antenv.axon_hooks" in sys.modules:
        return
    try:
        from trn_agent_boot.trn_boot import _ntff_profile_via_ctypes

        hook = _ntff_profile_via_ctypes("/opt/axon/libaxon_pjrt.so")
    except Exception:
        hook = None
    mod = types.ModuleType("antenv.axon_hooks")
    mod.get_axon_ntff_profile_hook = lambda: hook
    mod.set_axon_ntff_profile_hook = lambda h: None
    sys.modules["antenv.axon_hooks"] = mod


_install_ntff_hook()

PIPE_DEPTH = 1
ROPE_K_ENGINE = "gpsimd"   # "gpsimd" | "vector"
Y_DIVIDE = False           # divide needs 2 PSUM reads (illegal); recip+mul
N_OCOPY_DVE = 2            # of 4 out-proj pair copies per g on DVE (rest ACT)


def build_nc():
    nc = bacc.Bacc(None, target_bir_lowering=False, debug=False)

    xt = nc.declare_dram_parameter("xt", [128, (BT // TC) * KC * TC], BF16, isOutput=False)
    wqkv = nc.declare_dram_parameter("wqkv", [128, KC * 384], BF16, isOutput=False)
    wout = nc.declare_dram_parameter("wout", [128, C], BF16, isOutput=False)
    cs = nc.declare_dram_parameter("cs", [128, 4 * T], BF16, isOutput=False)
    outT = nc.declare_dram_parameter("outT", [C, BT], BF16, isOutput=True)

    with tile.TileContext(nc) as tc:
        with (
            tc.sbuf_pool(name="statics", bufs=1) as statics,
            tc.sbuf_pool(name="pool_x", bufs=3) as pool_x,
            tc.sbuf_pool(name="pool_rope", bufs=2) as pool_rope,
            tc.sbuf_pool(name="pool_qk", bufs=2) as pool_qk,
            tc.sbuf_pool(name="pool_v", bufs=2) as pool_v,
            tc.sbuf_pool(name="pool_y", bufs=2) as pool_y,
            tc.sbuf_pool(name="pool_vs", bufs=2) as pool_vs,
            tc.sbuf_pool(name="pool_p", bufs=4) as pool_p,
            tc.sbuf_pool(name="pool_o", bufs=4) as pool_o,
            tc.sbuf_pool(name="pool_rb", bufs=2) as pool_rb,
            tc.psum_pool(name="ps_mm", bufs=2) as ps_mm,
            tc.psum_pool(name="ps_st", bufs=2) as ps_st,
            tc.psum_pool(name="ps_y", bufs=2) as ps_y,
        ):
            ident = statics.tile([128, 128], BF16)
            make_identity(nc, ident)

            wqkv_sb = statics.tile([128, KC * 384], BF16)
            # split so the first chunks' matmuls aren't queued behind the
            # whole 0.8 MB table
            for q in range(4):
                qs = slice(q * 2 * 384, (q + 1) * 2 * 384)
                nc.sync.dma_start(out=wqkv_sb[:, qs], in_=wqkv[:, qs])
            # deferred statics: emitted after the first xt chunk's DMA so the
            # first QKV matmuls aren't queued behind the table loads
            wout_sb = statics.tile([128, C], BF16)
            cs_sb = statics.tile([128, 4, T], BF16)
            statics_emitted = []

            def emit_deferred_statics():
                if statics_emitted:
                    return
                statics_emitted.append(1)
                csv = cs[:, :].rearrange("p (f t) -> p f t", t=T)
                # column-chunked so rope(tci) only waits for its own chunk
                for tci in range(4):
                    sl = slice(tci * TC, (tci + 1) * TC)
                    nc.sync.dma_start(out=cs_sb[:, :, sl], in_=csv[:, :, sl])
                nc.sync.dma_start(out=wout_sb, in_=wout[:, :])

            def qkv_pair(b, pair, q_t, k_t, v_all):
                """QKV + RoPE for token chunks g0 = 4b+2*pair, g1 = g0+1.
                E/O stationaries stream both chunks per LDWEIGHTS."""
                va4 = v_all.rearrange("p (n h c) -> p n h c", h=2, c=128)
                g0 = 4 * b + 2 * pair
                xts = []
                for gi, g in enumerate((g0, g0 + 1)):
                    xt_sb = pool_x.tile([128, KC, TC], BF16, tag="x", name=f"xt_{g}")
                    nc.sync.dma_start(
                        out=xt_sb,
                        in_=xt[:, g * KC * TC : (g + 1) * KC * TC].rearrange(
                            "p (kc n) -> p kc n", n=TC
                        ),
                    )
                    xts.append(xt_sb)
                emit_deferred_statics()
                if pair == 0:
                    # ones columns of v_all (cols 64:128 of each head block)
                    nc.vector.memset(va4[:, :, :, 64:128], 1.0)

                # V per chunk (mm pool), then E and O streamed over the pair
                psvs = []
                for gi in range(2):
                    psv = ps_mm.tile([128, TC], F32, tag="mm", name=f"psv_{g0}_{gi}")
                    psvs.append(psv)
                    for kc in range(KC):
                        nc.tensor.matmul(
                            psv,
                            wqkv_sb[:, kc * 384 + 256 : kc * 384 + 384],
                            xts[gi][:, kc, :],
                            start=(kc == 0),
                            stop=(kc == KC - 1),
                        )
                ps_eos = [
                    ps_st.tile([128, 2, TC], F32, tag="st", name=f"eo_{g0}_{gi}")
                    for gi in range(2)
                ]
                for mi in range(2):  # 0 = E, 1 = O
                    for kc in range(KC):
                        for gi in range(2):
                            nc.tensor.matmul(
                                ps_eos[gi][:, mi, :],
                                wqkv_sb[
                                    :, kc * 384 + 128 * mi : kc * 384 + 128 * (mi + 1)
                                ],
                                xts[gi][:, kc, :],
                                start=(kc == 0),
                                stop=(kc == KC - 1),
                            )

                for gi, g in enumerate((g0, g0 + 1)):
                    tci = 2 * pair + gi
                    # v: PSUM -> SBUF bf16; PE-transpose the four 128-blocks
                    # into one [128, 4x128] bf16 PSUM tile; single 4D scatter
                    v_sb = pool_vs.tile([128, TC], BF16, tag="vs", name=f"vsb_{g}")
                    nc.scalar.activation(
                        out=v_sb, in_=psvs[gi],
                        func=mybir.ActivationFunctionType.Copy,
                    )
                    tr = ps_mm.tile([128, 4, 128], BF16, tag="mm", name=f"tr_{g}")
                    for s in range(4):
                        nc.tensor.transpose(
                            tr[:, s, :], v_sb[:, 128 * s : 128 * (s + 1)], ident
                        )
                    nc.vector.tensor_copy(
                        out=va4[:, 4 * tci : 4 * tci + 4, :, 0:64],
                        in_=tr.rearrange("p s (h c) -> p s h c", h=2),
                    )

                    # RoPE: two merged PSUM multiplies; m1 = [E*c; O*s],
                    # m2 = [E*s; O*c] against the [c|s|s|c] table
                    sl = slice(tci * TC, (tci + 1) * TC)
                    m1 = pool_rope.tile([128, 2, TC], BF16, tag="m1", name=f"m1_{g}")
                    nc.vector.tensor_mul(out=m1, in0=ps_eos[gi], in1=cs_sb[:, 0:2, sl])
                    m2 = pool_rope.tile([128, 2, TC], BF16, tag="m2", name=f"m2_{g}")
                    nc.vector.tensor_mul(out=m2, in0=ps_eos[gi], in1=cs_sb[:, 2:4, sl])

                    keng = nc.gpsimd if ROPE_K_ENGINE == "gpsimd" else nc.vector
                    # q_t rows [h0e|h0o|h1e|h1o]; E rows [q0e|q1e|k0e|k1e]
                    for h in range(2):
                        he = slice(32 * h, 32 * (h + 1))
                        nc.vector.tensor_sub(
                            out=q_t[64 * h : 64 * h + 32, sl],
                            in0=m1[he, 0, :], in1=m1[he, 1, :],
                        )
                        nc.vector.tensor_add(
                            out=q_t[64 * h + 32 : 64 * h + 64, sl],
                            in0=m2[he, 0, :], in1=m2[he, 1, :],
                        )
                        ke_ = slice(64 + 32 * h, 64 + 32 * (h + 1))
                        keng.tensor_sub(
                            out=k_t[64 * h : 64 * h + 32, sl],
                            in0=m1[ke_, 0, :], in1=m1[ke_, 1, :],
                        )
                        keng.tensor_add(
                            out=k_t[64 * h + 32 : 64 * h + 64, sl],
                            in0=m2[ke_, 0, :], in1=m2[ke_, 1, :],
                        )

            def attention_iter(b, i, q_t, k_t, v_all, y_t, pending_op):
                """pending_op: previous iteration's out-proj args, emitted
                after this iteration's first scores so its PSUM tiles queue
                behind ours in the st rotation (keeps the PE fed during the
                previous iteration's recip/ymul drain)."""
                nj = 4 * i + 4
                yaccs = {}
                for h in range(2):
                    yaccs[h] = ps_y.tile(
                        [128, TC], F32, tag="y", name=f"yacc_{b}_{i}_{h}"
                    )

                def st_of(j):
                    r = j - 4 * i
                    return 128 * r if r > 0 else 0

                # software-pipelined by one step: PE issues S(j),S(j),
                # PV(j-1),PV(j-1) back-to-back while exp(j) runs on ACT
                p_tiles = {}
                for j in range(nj + PIPE_DEPTH):
                    if j == 1 and pending_op is not None:
                        outproj_chunk(*pending_op)
                    if j < nj:
                        st = st_of(j)
                        r = j - 4 * i
                        ksl = slice(128 * j, 128 * (j + 1))
                        qsl = slice(TC * i + st, TC * (i + 1))
                        ps_s = ps_st.tile(
                            [128, 2, TC], F32, tag="st", name=f"s_{b}_{i}_{j}"
                        )
                        for h in range(2):
                            hs = slice(64 * h, 64 * (h + 1))
                            nc.tensor.matmul(
                                ps_s[:, h, st:], k_t[hs, ksl], q_t[hs, qsl],
                                start=True, stop=True,
                            )
                        p_sb = pool_p.tile(
                            [128, 2, TC], BF16, tag="p", name=f"p_{b}_{i}_{j}"
                        )
                        p_tiles[j] = p_sb
                        # one exp covering both heads
                        nc.scalar.activation(
                            out=p_sb[:, :, st:], in_=ps_s[:, :, st:],
                            func=mybir.ActivationFunctionType.Exp,
                            scale=0.125,
                        )
                        if r >= 0:
                            for h in range(2):
                                nc.gpsimd.affine_select(
                                    out=p_sb[:, h, st : st + 128],
                                    in_=p_sb[:, h, st : st + 128],
                                    pattern=[[1, 128]],
                                    channel_multiplier=-1,
                                    base=0,
                                    compare_op=mybir.AluOpType.is_ge,
                                    fill=0.0,
                                )
                    if j >= PIPE_DEPTH:
                        jp = j - PIPE_DEPTH
                        st = st_of(jp)
                        p_prev = p_tiles.pop(jp)
                        for h in range(2):
                            nc.tensor.matmul(
                                yaccs[h][:, st:],
                                v_all[
                                    :, VB * jp + 128 * h : VB * jp + 128 * (h + 1)
                                ],
                                p_prev[:, h, st:],
                                start=(jp == 0),
                                stop=(jp == nj - 1),
                            )
                ysl = slice(TC * i, TC * (i + 1))
                for h in range(2):
                    if Y_DIVIDE:
                        nc.vector.tensor_tensor(
                            out=y_t[64 * h : 64 * (h + 1), ysl],
                            in0=yaccs[h][0:64, :],
                            in1=yaccs[h][64:128, :],
                            op=mybir.AluOpType.divide,
                        )
                    else:
                        rb = pool_rb.tile(
                            [128, TC], F32, tag="rb", name=f"rb_{b}_{i}_{h}"
                        )
                        nc.vector.reciprocal_approx_fast(out=rb, in_=yaccs[h])
                        nc.vector.tensor_mul(
                            out=y_t[64 * h : 64 * (h + 1), ysl],
                            in0=yaccs[h][0:64, :],
                            in1=rb[64:128],
                        )
                return (b, y_t, i)

            def outproj_chunk(b, y_t, tci):
                    g = 4 * b + tci
                    for cp in range(4):
                        ps = ps_st.tile(
                            [128, 2, TC], F32, tag="st", name=f"op_{g}_{cp}"
                        )
                        for k in range(2):
                            cc = 2 * cp + k
                            nc.tensor.matmul(
                                ps[:, k, :],
                                wout_sb[:, 128 * cc : 128 * (cc + 1)],
                                y_t[:, TC * tci : TC * (tci + 1)],
                                start=True,
                                stop=True,
                            )
                        o_sb = pool_o.tile(
                            [128, 2, TC], BF16, tag="o", name=f"o_{g}_{cp}"
                        )
                        if cp < N_OCOPY_DVE:
                            nc.vector.tensor_copy(out=o_sb, in_=ps)
                        else:
                            nc.scalar.activation(
                                out=o_sb, in_=ps,
                                func=mybir.ActivationFunctionType.Copy,
                            )
                        nc.sync.dma_start(
                            out=outT[
                                256 * cp : 256 * (cp + 1), g * TC : (g + 1) * TC
                            ].rearrange("(j p) c -> p j c", j=2),
                            in_=o_sb,
                        )

            tiles = {}
            for b in range(B):
                tiles[b] = (
                    pool_qk.tile([128, T], BF16, tag="q", name=f"q_{b}"),
                    pool_qk.tile([128, T], BF16, tag="k", name=f"k_{b}"),
                    pool_v.tile([128, VB * NKJ], BF16, tag="v", name=f"v_{b}"),
                    pool_y.tile([128, T], BF16, tag="yt", name=f"y_{b}"),
                )

            q0, k0, v0, y0 = tiles[0]
            q1, k1, v1, y1 = tiles[1]
            qkv_pair(0, 0, q0, k0, v0)
            qkv_pair(0, 1, q0, k0, v0)
            # interleave b=1's projection into b=0's attention so its
            # DVE-heavy RoPE overlaps attention's PE-heavy stretch
            pend = attention_iter(0, 0, q0, k0, v0, y0, None)
            qkv_pair(1, 0, q1, k1, v1)
            pend = attention_iter(0, 1, q0, k0, v0, y0, pend)
            qkv_pair(1, 1, q1, k1, v1)
            pend = attention_iter(0, 2, q0, k0, v0, y0, pend)
            pend = attention_iter(0, 3, q0, k0, v0, y0, pend)
            for i in range(NQI):
                pend = attention_iter(1, i, q1, k1, v1, y1, pend)
            outproj_chunk(*pend)

    nc.compile()
    return nc


_NC_CACHE = None


def _get_nc():
    global _NC_CACHE
    if _NC_CACHE is None:
        _NC_CACHE = build_nc()
    return _NC_CACHE


def _host_prep(x, qkv_w, out_w):
    import ml_dtypes

    x = np.asarray(x, dtype=np.float32)
    qkv_w = np.asarray(qkv_w, dtype=np.float32)
    out_w = np.asarray(out_w, dtype=np.float32)

    # xt[p, ((g*KC)+kc)*TC + n] = x[g*TC + n, kc*128 + p] — one contiguous
    # line per (partition, chunk) for the per-chunk DMA
    xt = np.ascontiguousarray(
        x.reshape(BT // TC, TC, KC, 128).transpose(3, 0, 2, 1).reshape(128, -1)
    ).astype(ml_dtypes.bfloat16)

    # rope tables: row p uses frequency index p % 32; layout [c|s|s|c]
    t_idx = np.arange(T, dtype=np.float64)
    inv_freq = 1.0 / (10000.0 ** (np.arange(0, D, 2, dtype=np.float64) / D))  # 32
    ang = np.outer(np.tile(inv_freq, 4), t_idx)  # [128, T]
    cosa = np.cos(ang)
    sina = np.sin(ang)
    cs = np.concatenate([cosa, sina, sina, cosa], axis=1).astype(ml_dtypes.bfloat16)

    in_maps = []
    for core in range(N_CORES):
        h0 = 2 * core
        h1 = h0 + 1
        ev = np.arange(0, D, 2)
        od = np.arange(1, D, 2)
        e_rows = np.concatenate(
            [h0 * D + ev, h1 * D + ev, C + h0 * D + ev, C + h1 * D + ev]
        )
        o_rows = np.concatenate(
            [h0 * D + od, h1 * D + od, C + h0 * D + od, C + h1 * D + od]
        )
        v_rows = np.concatenate(
            [2 * C + h0 * D + np.arange(D), 2 * C + h1 * D + np.arange(D)]
        )
        rows = np.concatenate([e_rows, o_rows, v_rows])  # [384]
        w_part = qkv_w[rows]  # [384, C]
        # wqkv[p, kc*384 + m] = w_part[m, kc*128 + p]
        wqkv_c = np.ascontiguousarray(
            w_part.T.reshape(KC, 128, 384).transpose(1, 0, 2).reshape(128, KC * 384)
        ).astype(ml_dtypes.bfloat16)
        cols = np.concatenate([h0 * D + np.arange(D), h1 * D + np.arange(D)])
        wout_c = np.ascontiguousarray(out_w[:, cols].T).astype(
            ml_dtypes.bfloat16
        )  # [128, C]
        in_maps.append({"xt": xt, "wqkv": wqkv_c, "wout": wout_c, "cs": cs})
    return in_maps


def _run(in_maps, trace=False):
    nc = _get_nc()
    return run_bass_kernel_spmd(
        nc, in_maps, core_ids=list(range(N_CORES)), trace=trace
    )


def kernel(x, qkv_w, out_w, _trace=False, _results_box=None):
    in_maps = _host_prep(x, qkv_w, out_w)
    res = _run(in_maps, trace=_trace)
    if _results_box is not None:
        _results_box.append(res)
    acc = np.zeros((C, BT), np.float32)
    for r in res.results:
        acc += r["outT"].astype(np.float32)
    out = acc.T.reshape(B, T, C)
    return np.ascontiguousarray(out)


# revision 28
# speedup vs baseline: 1.2195x; 1.0052x over previous
"""Causal self-attention with RoPE (B=2, T=2048, C=1024, H=16, D=64) on 8
Trainium2 NeuronCores.

Sharding: tensor-parallel over heads — each core owns 2 heads (QKV and output
projections sliced on the head axis); the per-core partial outputs (full
[C, B*T] each) are summed on the host.

v3: engine-op-count-driven schedule. All matmuls bf16. QKV streams E/O
stationaries over g-pairs (halves their LDWEIGHTS). RoPE does two merged
[128,2,TC] PSUM multiplies against a 4-way [cos|sin|sin|cos] table plus the
8-way scatter add/subs (q on DVE, k on gpsimd). One exp per (i,j) covers both
heads. V transposed on the PE into a single [128,4x128] bf16 PSUM tile,
scattered with one 4D DVE copy per chunk. Softmax normalization is a single
tensor_tensor divide per head. Out-projection computes cc-pairs into
[128,2,TC] PSUM tiles, one drain copy each (DVE/ACT split), one [256,TC] DMA
each. qkv(b=1) is emitted interleaved into attention(b=0) so its DVE-heavy
RoPE overlaps attention's PE-heavy stretch.

Per-core layout (everything transposed: features on partitions, tokens free):
  xT [1024, 4096]     x^T, shared by all cores (bf16)
  QKV proj            qkv^T chunks via PE matmul (bf16), W rows pre-permuted
                      on host into three 128-row groups:
                        E = [q_h0_even(32) | q_h1_even | k_h0_even | k_h1_even]
                        O = same rows, odd dims
                        V = [v_h0(64) | v_h1(64)]
  RoPE                m1 = [E*c; O*s], m2 = [E*s; O*c]; rot pieces scattered
                      into head-contiguous q_t/k_t [128, T] with row layout
                      [h0_e(32) | h0_o(32) | h1_e(32) | h1_o(32)]
  scores^T            S^T[kj,qi] = k_t[h].T @ q_t[h], K=64, into a merged
                      [128, 2, TC] PSUM tile (diag chunks sliced)
  softmax             exp on ScalarE (scale=1/8 folded), causal triangle via
                      gpsimd affine_select, col sums via 64 ones-columns in v
  PV                  y^T[d,qi] accumulated over kj chunks; y = num/den via
                      one DVE divide per head
  out proj            outT[c,t] partial = woutT . y^T, bf16, cc-pair DMAs

Host gathers the 8 partial outT [1024, 4096] tensors, sums, transposes.
"""

import sys
import types

import numpy as np

import concourse.bass as bass
import concourse.tile as tile
from concourse import bacc
from concourse import mybir
from concourse.bass_utils import run_bass_kernel_spmd
from concourse.masks import make_identity

F32 = mybir.dt.float32
BF16 = mybir.dt.bfloat16

B = 2
T = 2048
C = 1024
D = 64
N_CORES = 8
BT = B * T              # 4096
TC = 512                # token chunk (free dim of most matmuls)
NQI = T // TC           # 4 qi chunks per batch
NKJ = T // 128          # 16 kj chunks per batch
KC = C // 128           # 8 contraction chunks for the projections
VB = 256                # v_all cols per kj block: h0 (d|ones) | h1 (d|ones)


def _install_ntff_hook():
    """bass_utils imports antenv.axon_hooks when tracing; this image lacks it.
    Recreate it from the ctypes NTFF driver so trace=True works."""
    if "antenv.axon_hooks" in sys.modules:
        return
    try:
        from trn_agent_boot.trn_boot import _ntff_profile_via_ctypes

        hook = _ntff_profile_via_ctypes("/opt/axon/libaxon_pjrt.so")
    except Exception:
        hook = None
    mod = types.ModuleType("antenv.axon_hooks")
    mod.get_axon_ntff_profile_hook = lambda: hook
    mod.set_axon_ntff_profile_hook = lambda h: None
    sys.modules["antenv.axon_hooks"] = mod


_install_ntff_hook()

PIPE_DEPTH = 1
ROPE_K_ENGINE = "gpsimd"   # "gpsimd" | "vector"
Y_DIVIDE = False           # divide needs 2 PSUM reads (illegal); recip+mul
N_OCOPY_DVE = 2            # of 4 out-proj pair copies per g on DVE (rest ACT)


def build_nc():
    nc = bacc.Bacc(None, target_bir_lowering=False, debug=False)

    xt = nc.declare_dram_parameter("xt", [128, (BT // TC) * KC * TC], BF16, isOutput=False)
    wqkv = nc.declare_dram_parameter("wqkv", [128, KC * 384], BF16, isOutput=False)
    wout = nc.declare_dram_parameter("wout", [128, C], BF16, isOutput=False)
    cs = nc.declare_dram_parameter("cs", [128, 4 * T], BF16, isOutput=False)
    outT = nc.declare_dram_parameter("outT", [C, BT], BF16, isOutput=True)

    with tile.TileContext(nc) as tc:
        with (
            tc.sbuf_pool(name="statics", bufs=1) as statics,
            tc.sbuf_pool(name="pool_x", bufs=3) as pool_x,
            tc.sbuf_pool(name="pool_rope", bufs=2) as pool_rope,
            tc.sbuf_pool(name="pool_qk", bufs=2) as pool_qk,
            tc.sbuf_pool(name="pool_v", bufs=2) as pool_v,
            tc.sbuf_pool(name="pool_y", bufs=2) as pool_y,
            tc.sbuf_pool(name="pool_vs", bufs=2) as pool_vs,
            tc.sbuf_pool(name="pool_p", bufs=4) as pool_p,
            tc.sbuf_pool(name="pool_o", bufs=4) as pool_o,
            tc.sbuf_pool(name="pool_rb", bufs=2) as pool_rb,
            tc.psum_pool(name="ps_mm", bufs=2) as ps_mm,
            tc.psum_pool(name="ps_st", bufs=2) as ps_st,
            tc.psum_pool(name="ps_y", bufs=2) as ps_y,
        ):
            ident = statics.tile([128, 128], BF16)
            make_identity(nc, ident)

            wqkv_sb = statics.tile([128, KC * 384], BF16)
            # split so the first chunks' matmuls aren't queued behind the
            # whole 0.8 MB table
            for q in range(4):
                qs = slice(q * 2 * 384, (q + 1) * 2 * 384)
                nc.sync.dma_start(out=wqkv_sb[:, qs], in_=wqkv[:, qs])
            # deferred statics: emitted after the first xt chunk's DMA so the
            # first QKV matmuls aren't queued behind the table loads
            wout_sb = statics.tile([128, C], BF16)
            cs_sb = statics.tile([128, 4, T], BF16)
            statics_emitted = []

            def emit_deferred_statics():
                if statics_emitted:
                    return
                statics_emitted.append(1)
                csv = cs[:, :].rearrange("p (f t) -> p f t", t=T)
                # column-chunked so rope(tci) only waits for its own chunk;
                # on the scalar queue so they don't delay xt loads on sync
                for tci in range(4):
                    sl = slice(tci * TC, (tci + 1) * TC)
                    nc.scalar.dma_start(out=cs_sb[:, :, sl], in_=csv[:, :, sl])
                nc.scalar.dma_start(out=wout_sb, in_=wout[:, :])

            def qkv_pair(b, pair, q_t, k_t, v_all, k_eng=None):
                """QKV + RoPE for token chunks g0 = 4b+2*pair, g1 = g0+1.
                E/O stationaries stream both chunks per LDWEIGHTS."""
                va4 = v_all.rearrange("p (n h c) -> p n h c", h=2, c=128)
                g0 = 4 * b + 2 * pair
                xts = []
                for gi, g in enumerate((g0, g0 + 1)):
                    xt_sb = pool_x.tile([128, KC, TC], BF16, tag="x", name=f"xt_{g}")
                    xv = xt[:, g * KC * TC : (g + 1) * KC * TC].rearrange(
                        "p (kc n) -> p kc n", n=TC
                    )
                    # split halves across the two hwdge queues
                    nc.sync.dma_start(out=xt_sb[:, 0:4, :], in_=xv[:, 0:4, :])
                    nc.scalar.dma_start(out=xt_sb[:, 4:8, :], in_=xv[:, 4:8, :])
                    xts.append(xt_sb)
                emit_deferred_statics()
                if pair == 0:
                    # ones columns of v_all (cols 64:128 of each head block)
                    nc.vector.memset(va4[:, :, :, 64:128], 1.0)

                # V per chunk (mm pool), then E and O streamed over the pair
                psvs = []
                for gi in range(2):
                    psv = ps_mm.tile([128, TC], F32, tag="mm", name=f"psv_{g0}_{gi}")
                    psvs.append(psv)
                    for kc in range(KC):
                        nc.tensor.matmul(
                            psv,
                            wqkv_sb[:, kc * 384 + 256 : kc * 384 + 384],
                            xts[gi][:, kc, :],
                            start=(kc == 0),
                            stop=(kc == KC - 1),
                        )
                ps_eos = [
                    ps_st.tile([128, 2, TC], F32, tag="st", name=f"eo_{g0}_{gi}")
                    for gi in range(2)
                ]
                for mi in range(2):  # 0 = E, 1 = O
                    for kc in range(KC):
                        for gi in range(2):
                            nc.tensor.matmul(
                                ps_eos[gi][:, mi, :],
                                wqkv_sb[
                                    :, kc * 384 + 128 * mi : kc * 384 + 128 * (mi + 1)
                                ],
                                xts[gi][:, kc, :],
                                start=(kc == 0),
                                stop=(kc == KC - 1),
                            )

                for gi, g in enumerate((g0, g0 + 1)):
                    tci = 2 * pair + gi
                    # v: PSUM -> SBUF bf16; PE-transpose the four 128-blocks
                    # into one [128, 4x128] bf16 PSUM tile; single 4D scatter
                    v_sb = pool_vs.tile([128, TC], BF16, tag="vs", name=f"vsb_{g}")
                    nc.scalar.activation(
                        out=v_sb, in_=psvs[gi],
                        func=mybir.ActivationFunctionType.Copy,
                    )
                    tr = ps_mm.tile([128, 4, 128], BF16, tag="mm", name=f"tr_{g}")
                    for s in range(4):
                        nc.tensor.transpose(
                            tr[:, s, :], v_sb[:, 128 * s : 128 * (s + 1)], ident
                        )
                    nc.vector.tensor_copy(
                        out=va4[:, 4 * tci : 4 * tci + 4, :, 0:64],
                        in_=tr.rearrange("p s (h c) -> p s h c", h=2),
                    )

                    # RoPE: two merged PSUM multiplies; m1 = [E*c; O*s],
                    # m2 = [E*s; O*c] against the [c|s|s|c] table
                    sl = slice(tci * TC, (tci + 1) * TC)
                    m1 = pool_rope.tile([128, 2, TC], BF16, tag="m1", name=f"m1_{g}")
                    nc.vector.tensor_mul(out=m1, in0=ps_eos[gi], in1=cs_sb[:, 0:2, sl])
                    m2 = pool_rope.tile([128, 2, TC], BF16, tag="m2", name=f"m2_{g}")
                    nc.vector.tensor_mul(out=m2, in0=ps_eos[gi], in1=cs_sb[:, 2:4, sl])

                    keng = k_eng or (
                        nc.gpsimd if ROPE_K_ENGINE == "gpsimd" else nc.vector
                    )
                    # q_t rows [h0e|h0o|h1e|h1o]; E rows [q0e|q1e|k0e|k1e]
                    for h in range(2):
                        he = slice(32 * h, 32 * (h + 1))
                        nc.vector.tensor_sub(
                            out=q_t[64 * h : 64 * h + 32, sl],
                            in0=m1[he, 0, :], in1=m1[he, 1, :],
                        )
                        nc.vector.tensor_add(
                            out=q_t[64 * h + 32 : 64 * h + 64, sl],
                            in0=m2[he, 0, :], in1=m2[he, 1, :],
                        )
                        ke_ = slice(64 + 32 * h, 64 + 32 * (h + 1))
                        keng.tensor_sub(
                            out=k_t[64 * h : 64 * h + 32, sl],
                            in0=m1[ke_, 0, :], in1=m1[ke_, 1, :],
                        )
                        keng.tensor_add(
                            out=k_t[64 * h + 32 : 64 * h + 64, sl],
                            in0=m2[ke_, 0, :], in1=m2[ke_, 1, :],
                        )

            def attention_iter(b, i, q_t, k_t, v_all, y_t, pending_op):
                """pending_op: previous iteration's out-proj args, emitted
                after this iteration's first scores so its PSUM tiles queue
                behind ours in the st rotation (keeps the PE fed during the
                previous iteration's recip/ymul drain)."""
                nj = 4 * i + 4
                yaccs = {}
                for h in range(2):
                    yaccs[h] = ps_y.tile(
                        [128, TC], F32, tag="y", name=f"yacc_{b}_{i}_{h}"
                    )

                def st_of(j):
                    r = j - 4 * i
                    return 128 * r if r > 0 else 0

                # software-pipelined by one step: PE issues S(j),S(j),
                # PV(j-1),PV(j-1) back-to-back while exp(j) runs on ACT
                p_tiles = {}
                for j in range(nj + PIPE_DEPTH):
                    if j == 1 and pending_op is not None:
                        outproj_chunk(*pending_op)
                    if j < nj:
                        st = st_of(j)
                        r = j - 4 * i
                        ksl = slice(128 * j, 128 * (j + 1))
                        qsl = slice(TC * i + st, TC * (i + 1))
                        ps_s = ps_st.tile(
                            [128, 2, TC], F32, tag="st", name=f"s_{b}_{i}_{j}"
                        )
                        for h in range(2):
                            hs = slice(64 * h, 64 * (h + 1))
                            nc.tensor.matmul(
                                ps_s[:, h, st:], k_t[hs, ksl], q_t[hs, qsl],
                                start=True, stop=True,
                            )
                        p_sb = pool_p.tile(
                            [128, 2, TC], BF16, tag="p", name=f"p_{b}_{i}_{j}"
                        )
                        p_tiles[j] = p_sb
                        # one exp covering both heads
                        nc.scalar.activation(
                            out=p_sb[:, :, st:], in_=ps_s[:, :, st:],
                            func=mybir.ActivationFunctionType.Exp,
                            scale=0.125,
                        )
                        if r >= 0:
                            for h in range(2):
                                nc.gpsimd.affine_select(
                                    out=p_sb[:, h, st : st + 128],
                                    in_=p_sb[:, h, st : st + 128],
                                    pattern=[[1, 128]],
                                    channel_multiplier=-1,
                                    base=0,
                                    compare_op=mybir.AluOpType.is_ge,
                                    fill=0.0,
                                )
                    if j >= PIPE_DEPTH:
                        jp = j - PIPE_DEPTH
                        st = st_of(jp)
                        p_prev = p_tiles.pop(jp)
                        for h in range(2):
                            nc.tensor.matmul(
                                yaccs[h][:, st:],
                                v_all[
                                    :, VB * jp + 128 * h : VB * jp + 128 * (h + 1)
                                ],
                                p_prev[:, h, st:],
                                start=(jp == 0),
                                stop=(jp == nj - 1),
                            )
                ysl = slice(TC * i, TC * (i + 1))
                for h in range(2):
                    if Y_DIVIDE:
                        nc.vector.tensor_tensor(
                            out=y_t[64 * h : 64 * (h + 1), ysl],
                            in0=yaccs[h][0:64, :],
                            in1=yaccs[h][64:128, :],
                            op=mybir.AluOpType.divide,
                        )
                    else:
                        rb = pool_rb.tile(
                            [128, TC], F32, tag="rb", name=f"rb_{b}_{i}_{h}"
                        )
                        nc.vector.reciprocal_approx_fast(out=rb, in_=yaccs[h])
                        nc.vector.tensor_mul(
                            out=y_t[64 * h : 64 * (h + 1), ysl],
                            in0=yaccs[h][0:64, :],
                            in1=rb[64:128],
                        )
                return (b, y_t, i)

            def outproj_chunk(b, y_t, tci):
                    g = 4 * b + tci
                    for cp in range(4):
                        ps = ps_st.tile(
                            [128, 2, TC], F32, tag="st", name=f"op_{g}_{cp}"
                        )
                        for k in range(2):
                            cc = 2 * cp + k
                            nc.tensor.matmul(
                                ps[:, k, :],
                                wout_sb[:, 128 * cc : 128 * (cc + 1)],
                                y_t[:, TC * tci : TC * (tci + 1)],
                                start=True,
                                stop=True,
                            )
                        o_sb = pool_o.tile(
                            [128, 2, TC], BF16, tag="o", name=f"o_{g}_{cp}"
                        )
                        if cp < N_OCOPY_DVE:
                            nc.vector.tensor_copy(out=o_sb, in_=ps)
                        else:
                            nc.scalar.activation(
                                out=o_sb, in_=ps,
                                func=mybir.ActivationFunctionType.Copy,
                            )
                        deng = nc.sync if cp % 2 == 0 else nc.scalar
                        deng.dma_start(
                            out=outT[
                                256 * cp : 256 * (cp + 1), g * TC : (g + 1) * TC
                            ].rearrange("(j p) c -> p j c", j=2),
                            in_=o_sb,
                        )

            tiles = {}
            for b in range(B):
                tiles[b] = (
                    pool_qk.tile([128, T], BF16, tag="q", name=f"q_{b}"),
                    pool_qk.tile([128, T], BF16, tag="k", name=f"k_{b}"),
                    pool_v.tile([128, VB * NKJ], BF16, tag="v", name=f"v_{b}"),
                    pool_y.tile([128, T], BF16, tag="yt", name=f"y_{b}"),
                )

            q0, k0, v0, y0 = tiles[0]
            q1, k1, v1, y1 = tiles[1]
            qkv_pair(0, 0, q0, k0, v0)
            qkv_pair(0, 1, q0, k0, v0)
            # interleave b=1's projection into b=0's attention so its
            # DVE-heavy RoPE overlaps attention's PE-heavy stretch
            # k-adds on DVE here: gpsimd's queue must stay clear for b=0's
            # affine_selects (exp->affine->PV is on attention's critical path)
            pend = attention_iter(0, 0, q0, k0, v0, y0, None)
            qkv_pair(1, 0, q1, k1, v1, k_eng=nc.vector)
            pend = attention_iter(0, 1, q0, k0, v0, y0, pend)
            qkv_pair(1, 1, q1, k1, v1, k_eng=nc.vector)
            pend = attention_iter(0, 2, q0, k0, v0, y0, pend)
            pend = attention_iter(0, 3, q0, k0, v0, y0, pend)
            for i in range(NQI):
                pend = attention_iter(1, i, q1, k1, v1, y1, pend)
            outproj_chunk(*pend)

    nc.compile()
    return nc


_NC_CACHE = None


def _get_nc():
    global _NC_CACHE
    if _NC_CACHE is None:
        _NC_CACHE = build_nc()
    return _NC_CACHE


def _host_prep(x, qkv_w, out_w):
    import ml_dtypes

    x = np.asarray(x, dtype=np.float32)
    qkv_w = np.asarray(qkv_w, dtype=np.float32)
    out_w = np.asarray(out_w, dtype=np.float32)

    # xt[p, ((g*KC)+kc)*TC + n] = x[g*TC + n, kc*128 + p] — one contiguous
    # line per (partition, chunk) for the per-chunk DMA
    xt = np.ascontiguousarray(
        x.reshape(BT // TC, TC, KC, 128).transpose(3, 0, 2, 1).reshape(128, -1)
    ).astype(ml_dtypes.bfloat16)

    # rope tables: row p uses frequency index p % 32; layout [c|s|s|c]
    t_idx = np.arange(T, dtype=np.float64)
    inv_freq = 1.0 / (10000.0 ** (np.arange(0, D, 2, dtype=np.float64) / D))  # 32
    ang = np.outer(np.tile(inv_freq, 4), t_idx)  # [128, T]
    cosa = np.cos(ang)
    sina = np.sin(ang)
    cs = np.concatenate([cosa, sina, sina, cosa], axis=1).astype(ml_dtypes.bfloat16)

    in_maps = []
    for core in range(N_CORES):
        h0 = 2 * core
        h1 = h0 + 1
        ev = np.arange(0, D, 2)
        od = np.arange(1, D, 2)
        e_rows = np.concatenate(
            [h0 * D + ev, h1 * D + ev, C + h0 * D + ev, C + h1 * D + ev]
        )
        o_rows = np.concatenate(
            [h0 * D + od, h1 * D + od, C + h0 * D + od, C + h1 * D + od]
        )
        v_rows = np.concatenate(
            [2 * C + h0 * D + np.arange(D), 2 * C + h1 * D + np.arange(D)]
        )
        rows = np.concatenate([e_rows, o_rows, v_rows])  # [384]
        w_part = qkv_w[rows]  # [384, C]
        # wqkv[p, kc*384 + m] = w_part[m, kc*128 + p]
        wqkv_c = np.ascontiguousarray(
            w_part.T.reshape(KC, 128, 384).transpose(1, 0, 2).reshape(128, KC * 384)
        ).astype(ml_dtypes.bfloat16)
        cols = np.concatenate([h0 * D + np.arange(D), h1 * D + np.arange(D)])
        wout_c = np.ascontiguousarray(out_w[:, cols].T).astype(
            ml_dtypes.bfloat16
        )  # [128, C]
        in_maps.append({"xt": xt, "wqkv": wqkv_c, "wout": wout_c, "cs": cs})
    return in_maps


def _run(in_maps, trace=False):
    nc = _get_nc()
    return run_bass_kernel_spmd(
        nc, in_maps, core_ids=list(range(N_CORES)), trace=trace
    )


def kernel(x, qkv_w, out_w, _trace=False, _results_box=None):
    in_maps = _host_prep(x, qkv_w, out_w)
    res = _run(in_maps, trace=_trace)
    if _results_box is not None:
        _results_box.append(res)
    acc = np.zeros((C, BT), np.float32)
    for r in res.results:
        acc += r["outT"].astype(np.float32)
    out = acc.T.reshape(B, T, C)
    return np.ascontiguousarray(out)
